# revision 34
# baseline (speedup 1.0000x reference)
"""Distributed MQA attention kernel for 8 TRN2 NeuronCores.

Sharding: sequence-parallel over query rows. Core c owns query rows
[256c, 256(c+1)) of BOTH batches. All 16 heads stay local to each core, so
the output projection needs no cross-core reduction; the only collective is
a small AllGather of the shared (MQA) K^T / V projections.

Precision: logits have std ~2000 (un-normalized q·k), so softmax is
near-argmax: the q/k/scores path runs in true fp32 (4-pass matmuls).
attn@v and the output projection run in bf16.
"""

import sys

if "/opt/trn_rl_repo" not in sys.path:
    sys.path.insert(0, "/opt/trn_rl_repo")

import numpy as np

B = 2
N = 2048
D = 2048
H = 16
DH = 128
NCORES = 8
RPC = N // NCORES  # query rows per core, per batch (256)
EPS = 1e-5
SQRT_D = float(np.sqrt(np.float64(D)))

_PROGRAMS = {}


def _build_program(reps=1, ablate=()):
    ablate = set(ablate)
    from concourse import bacc, masks, mybir, tile

    f32 = mybir.dt.float32
    bf16 = mybir.dt.bfloat16
    Alu = mybir.AluOpType
    AFT = mybir.ActivationFunctionType

    nc = bacc.Bacc(
        "TRN2", target_bir_lowering=False, debug=False, num_devices=NCORES
    )

    x_d = nc.dram_tensor("x", (B, RPC, D), f32, kind="ExternalInput").ap()
    bias_d = nc.dram_tensor("attn_bias", (H, RPC, N), bf16, kind="ExternalInput").ap()
    wq_d = nc.dram_tensor("wq", (D, H * DH), f32, kind="ExternalInput").ap()
    wk_d = nc.dram_tensor("wk", (D, DH), f32, kind="ExternalInput").ap()
    wv_d = nc.dram_tensor("wv", (D, DH), f32, kind="ExternalInput").ap()
    wo_d = nc.dram_tensor("wo", (H * DH, D), bf16, kind="ExternalInput").ap()
    out_d = nc.dram_tensor("out", (B, RPC, D), f32, kind="ExternalOutput").ap()

    CT = D // 128  # 16 contraction tiles
    RB = (B * RPC) // 128  # 4 row blocks per core
    JT = N // 128  # 16 key tiles per batch

    with tile.TileContext(nc) as tc:
        with (
            tc.tile_pool(name="const", bufs=1) as const_pool,
            tc.tile_pool(name="persist", bufs=1) as persist,
            tc.tile_pool(name="xq", bufs=2) as xq_pool,
            tc.tile_pool(name="stat", bufs=12) as stat_pool,
            tc.tile_pool(name="bias", bufs=2) as bias_pool,
            tc.tile_pool(name="ebuf", bufs=2) as e_pool,
            tc.tile_pool(name="etbuf", bufs=2) as ets_pool,
            tc.tile_pool(name="vstg", bufs=1) as vstg_pool,
            tc.tile_pool(name="wo_s", bufs=2) as wo_pool,
            tc.tile_pool(name="otb", bufs=1) as ot_pool,
            tc.tile_pool(name="outb", bufs=2) as out_pool,
            tc.tile_pool(name="simps", bufs=2, space="PSUM") as sim_ps,
            tc.tile_pool(name="etps", bufs=1, space="PSUM") as et_ps,
            tc.tile_pool(name="shps", bufs=2, space="PSUM") as sh_ps,
            tc.tile_pool(name="dram", bufs=1, space="DRAM") as dram_pool,
        ):
            ident_f32 = const_pool.tile([128, 128], f32)
            ident_bf16 = const_pool.tile([128, 128], bf16)
            masks.make_identity(nc, ident_f32[:])
            masks.make_identity(nc, ident_bf16[:])

            wk_sb = persist.tile([128, CT, DH], f32)
            wv_sb = persist.tile([128, CT, DH], f32)
            nc.sync.dma_start(wk_sb[:], wk_d.rearrange("(ct p) d -> p ct d", p=128))
            nc.sync.dma_start(wv_sb[:], wv_d.rearrange("(ct p) d -> p ct d", p=128))

            sc_bf = "scores_bf16" in ablate
            uT = persist.tile([128, CT, B * RPC], f32)
            qT = persist.tile([128, CT, B * RPC], bf16 if sc_bf else f32)
            kT_full = persist.tile([128, B, N], f32)
            kT_use = persist.tile([128, B, N], bf16) if sc_bf else kT_full
            v_all = persist.tile([128, B * JT, DH], bf16)
            v_loc = persist.tile([128, RB, DH], f32)
            kT_loc = persist.tile([128, B * RPC], f32)

            bnc_k_in = dram_pool.tile([128, B * RPC], f32)
            bnc_k_out = dram_pool.tile([NCORES, 128, B * RPC], f32, addr_space="Shared")
            bnc_v_in = dram_pool.tile([B * RPC, DH], f32)
            bnc_v_out = dram_pool.tile([NCORES, B * RPC, DH], f32, addr_space="Shared")

            # ---- Stage 0: RMSNorm rows + transpose to u^T ------------------
            for b in range(B):
                for itile in range(RPC // 128):
                    rb = b * (RPC // 128) + itile
                    xt = xq_pool.tile([128, D], f32, tag="xq")
                    nc.sync.dma_start(xt[:], x_d[b, 128 * itile : 128 * (itile + 1), :])
                    scr = out_pool.tile([128, D], f32, tag="out_sb")
                    ssq = stat_pool.tile([128, 1], f32)
                    nc.scalar.activation(scr[:], xt[:], AFT.Square, accum_out=ssq[:])
                    tb = stat_pool.tile([128, 1], f32)
                    nc.vector.tensor_scalar_add(tb[:], ssq[:], EPS)
                    sq = stat_pool.tile([128, 1], f32)
                    nc.scalar.sqrt(sq[:], tb[:])
                    r0 = stat_pool.tile([128, 1], f32)
                    nc.vector.reciprocal(r0[:], sq[:])
                    # one Newton step for rsqrt accuracy: y = r0*(1.5-0.5*t*r0^2)
                    hh = stat_pool.tile([128, 1], f32)
                    nc.vector.tensor_mul(hh[:], r0[:], r0[:])
                    nc.vector.tensor_mul(hh[:], hh[:], tb[:])
                    h2 = stat_pool.tile([128, 1], f32)
                    nc.vector.tensor_scalar(h2[:], hh[:], -0.5, 1.5, Alu.mult, Alu.add)
                    sc = stat_pool.tile([128, 1], f32)
                    nc.vector.tensor_mul(sc[:], r0[:], h2[:])
                    nc.vector.tensor_scalar_mul(sc[:], sc[:], SQRT_D)
                    nc.vector.tensor_scalar_mul(xt[:], xt[:], sc[:])
                    for cg in range(4):
                        trp = sh_ps.tile([128, 512], f32, tag="shps")
                        for k in range(4):
                            ct = 4 * cg + k
                            nc.tensor.transpose(
                                trp[:, 128 * k : 128 * (k + 1)],
                                xt[:, 128 * ct : 128 * (ct + 1)],
                                ident_f32[:],
                            )
                        nc.vector.tensor_copy(
                            uT[:, 4 * cg : 4 * cg + 4, 128 * rb : 128 * (rb + 1)],
                            trp.rearrange("p (k r) -> p k r", k=4),
                        )

            # ---- Stage 1a: local k^T / v projections + AllGather -----------
            kps = sh_ps.tile([128, 512], f32, tag="shps")
            for ct in range(CT):
                nc.tensor.matmul(
                    kps[:], wk_sb[:, ct, :], uT[:, ct, :],
                    start=(ct == 0), stop=(ct == CT - 1),
                )
            nc.vector.tensor_copy(kT_loc[:], kps[:])
            nc.gpsimd.dma_start(bnc_k_in[:], kT_loc[:])

            for rb in range(RB):
                vps = sh_ps.tile([128, 128], f32, tag="shps")
                for ct in range(CT):
                    nc.tensor.matmul(
                        vps[:],
                        uT[:, ct, 128 * rb : 128 * (rb + 1)],
                        wv_sb[:, ct, :],
                        start=(ct == 0), stop=(ct == CT - 1),
                    )
                nc.vector.tensor_copy(v_loc[:, rb, :], vps[:])
            nc.gpsimd.dma_start(
                bnc_v_in.rearrange("(s p) d -> p s d", p=128), v_loc[:]
            )

            if "nocc" not in ablate:
                nc.gpsimd.collective_compute(
                    "AllGather", Alu.bypass,
                    replica_groups=[list(range(NCORES))],
                    ins=[bnc_k_in[:].opt()], outs=[bnc_k_out[:].opt()],
                )
                nc.gpsimd.collective_compute(
                    "AllGather", Alu.bypass,
                    replica_groups=[list(range(NCORES))],
                    ins=[bnc_v_in[:].opt()], outs=[bnc_v_out[:].opt()],
                )
            else:
                nc.gpsimd.dma_start(bnc_k_out[0], bnc_k_in[:])
                nc.gpsimd.dma_start(bnc_v_out[0], bnc_v_in[:])

            # ---- Stage 1b: q^T projection (fp32) ---------------------------
            for mt in range(H):
                wqt = xq_pool.tile([128, CT, 128], f32, tag="xq")
                nc.sync.dma_start(
                    wqt[:],
                    wq_d[:, 128 * mt : 128 * (mt + 1)].rearrange(
                        "(ct p) m -> p ct m", p=128
                    ),
                )
                qps = sh_ps.tile([128, 512], f32, tag="shps")
                for ct in range(CT):
                    nc.tensor.matmul(
                        qps[:], wqt[:, ct, :], uT[:, ct, :],
                        start=(ct == 0), stop=(ct == CT - 1),
                    )
                nc.vector.tensor_copy(qT[:, mt, :], qps[:])

            # ---- gather-in: k^T full + v full ------------------------------
            # bounce layout: core g contributed cols [b0 rows | b1 rows]
            nc.sync.dma_start(
                kT_full.rearrange("p b (g r) -> p b g r", g=NCORES),
                bnc_k_out.rearrange("g p (b r) -> p b g r", b=B),
            )
            if sc_bf:
                nc.vector.tensor_copy(kT_use[:], kT_full[:])
            for g in range(NCORES):
                vst = vstg_pool.tile([128, RB, DH], f32)
                nc.sync.dma_start(
                    vst[:], bnc_v_out[g].rearrange("(s p) d -> p s d", p=128)
                )
                for b in range(B):
                    nc.vector.tensor_copy(
                        v_all[:, b * JT + 2 * g : b * JT + 2 * g + 2, :],
                        vst[:, 2 * b : 2 * b + 2, :],
                    )

            # ---- Stage 2: attention + output projection --------------------
            def s2_front(itile, h, b, bt):
                """scores + softmax for one (itile, h, b); returns et2."""
                i0 = b * RPC + 128 * itile
                HW_ = N // 2  # half width (1024)
                et = e_pool.tile([128, N], bf16, tag="et")
                et2 = e_pool.tile([128, N], bf16, tag="et2")
                nms = stat_pool.tile([128, 2], f32, tag="nms")
                for hf in range(2):
                    simh = sim_ps.tile([128, HW_], f32, tag="sim")
                    for jb in range(2):
                        j0 = HW_ * hf + 512 * jb
                        nc.tensor.matmul(
                            simh[:, 512 * jb : 512 * (jb + 1)],
                            qT[:, h, i0 : i0 + 128],
                            kT_use[:, b, j0 : j0 + 512],
                            start=True, stop=True,
                        )
                    # nms[:, hf] = -max(sim_half)
                    nc.vector.tensor_reduce(
                        out=nms[:, hf : hf + 1], in_=simh[:],
                        op=Alu.max, axis=mybir.AxisListType.X, negate=True,
                    )
                    nc.scalar.activation(
                        et[:, HW_ * hf : HW_ * (hf + 1)], simh[:],
                        AFT.Exp, bias=nms[:, hf : hf + 1], scale=1.0,
                    )
                # combined -max = min over halves; per-half rescale
                # corr_h = exp(nm - nm_h) folded into the stt scalar
                nm = stat_pool.tile([128, 1], f32, tag="nm")
                nc.vector.tensor_reduce(
                    out=nm[:], in_=nms[:], op=Alu.min, axis=mybir.AxisListType.X
                )
                dh = stat_pool.tile([128, 2], f32, tag="dh")
                nc.vector.tensor_scalar(
                    dh[:], nms[:], -1.0, nm[:], Alu.mult, Alu.add
                )
                corr = stat_pool.tile([128, 2], f32, tag="corr")
                nc.scalar.activation(corr[:], dh[:], AFT.Exp)
                rs2 = stat_pool.tile([128, 2], f32, tag="rs2")
                for hf in range(2):
                    nc.vector.scalar_tensor_tensor(
                        out=et2[:, HW_ * hf : HW_ * (hf + 1)],
                        in0=et[:, HW_ * hf : HW_ * (hf + 1)],
                        scalar=corr[:, hf : hf + 1],
                        in1=bt[:, HW_ * hf : HW_ * (hf + 1)],
                        op0=Alu.mult, op1=Alu.mult,
                        accum_out=rs2[:, hf : hf + 1],
                    )
                rs = stat_pool.tile([128, 1], f32, tag="rs")
                nc.vector.tensor_reduce(
                    out=rs[:], in_=rs2[:], op=Alu.add, axis=mybir.AxisListType.X
                )
                rcp = stat_pool.tile([128, 1], f32, tag="rcp")
                nc.vector.reciprocal(rcp[:], rs[:])
                nc.vector.tensor_scalar_mul(et2[:], et2[:], rcp[:])
                return et2

            def s2_back(b, h, et2, oT):
                """E-transpose + attn@v for one finished front iteration."""
                etp = et_ps.tile([128, N], bf16, tag="etp")
                for jt in range(JT):
                    nc.tensor.transpose(
                        etp[:, 128 * jt : 128 * (jt + 1)],
                        et2[:, 128 * jt : 128 * (jt + 1)],
                        ident_bf16[:],
                    )
                ets = ets_pool.tile([128, N], bf16, tag="ets")
                nc.scalar.copy(ets[:], etp[:])
                otp = sh_ps.tile([128, 128], f32, tag="shps")
                for jt in range(JT):
                    nc.tensor.matmul(
                        otp[:],
                        v_all[:, b * JT + jt, :],
                        ets[:, 128 * jt : 128 * (jt + 1)],
                        start=(jt == 0), stop=(jt == JT - 1),
                    )
                nc.vector.tensor_copy(oT[:, b, h, :], otp[:])

            def stage2():
                for itile in range(RPC // 128):
                    oT = ot_pool.tile([128, B, H, 128], bf16, tag="oT")
                    iters = [(h, b) for h in range(H) for b in range(B)]
                    pending = None  # (b, h, et2) awaiting back-stage
                    bt = None
                    for idx in range(len(iters) + 1):
                        if idx < len(iters):
                            h, b = iters[idx]
                            if b == 0:
                                # bt holds exp(bias) (host-precomputed)
                                bt = bias_pool.tile([128, N], bf16, tag="bt")
                                nc.sync.dma_start(
                                    bt[:],
                                    bias_d[h, 128 * itile : 128 * (itile + 1), :],
                                )
                            et2 = s2_front(itile, h, b, bt)
                            nxt = (b, h, et2)
                        else:
                            nxt = None
                        if pending is not None:
                            s2_back(pending[0], pending[1], pending[2], oT)
                        pending = nxt
                    outs = []
                    for b in range(B):
                        ob = out_pool.tile([128, D], f32, tag="out_sb")
                        outs.append(ob)
                    for et_i in range(D // 256):
                        wot = wo_pool.tile([128, H, 256], bf16, tag="wot")
                        nc.sync.dma_start(
                            wot[:],
                            wo_d[:, 256 * et_i : 256 * (et_i + 1)].rearrange(
                                "(mt p) e -> p mt e", p=128
                            ),
                        )
                        for b in range(B):
                            fin = sh_ps.tile([128, 256], f32, tag="shps")
                            if "nowo" not in ablate:
                                for mt in range(H):
                                    nc.tensor.matmul(
                                        fin[:], oT[:, b, mt, :], wot[:, mt, :],
                                        start=(mt == 0), stop=(mt == H - 1),
                                    )
                            nc.vector.tensor_copy(
                                outs[b][:, 256 * et_i : 256 * (et_i + 1)], fin[:]
                            )
                    for b in range(B):
                        nc.sync.dma_start(
                            out_d[b, 128 * itile : 128 * (itile + 1), :], outs[b][:]
                        )

            if "skip2" in ablate:
                pass
            elif reps == 1:
                stage2()
            else:
                with tc.For_i(0, reps, 1):
                    stage2()

    nc.compile()
    return nc


def _get_program(reps=1, ablate=()):
    key = (reps, tuple(sorted(ablate)))
    if key not in _PROGRAMS:
        _PROGRAMS[key] = _build_program(reps, ablate)
    return _PROGRAMS[key]


def _make_in_maps(inputs):
    import ml_dtypes

    x = np.asarray(inputs["x"], np.float32)
    bias = np.asarray(inputs["attn_bias"], np.float32)
    gamma = np.asarray(inputs["gamma"], np.float32)
    wq = np.asarray(inputs["wq"], np.float32)
    wk = np.asarray(inputs["wk"], np.float32)
    wv = np.asarray(inputs["wv"], np.float32)
    wo = np.asarray(inputs["wo"], np.float32)

    g = gamma[:, None]
    wqp = np.ascontiguousarray(g * wq * np.float32(DH**-0.5), dtype=np.float32)
    wkp = np.ascontiguousarray(g * wk, dtype=np.float32)
    wvp = np.ascontiguousarray(g * wv, dtype=np.float32)
    wo_bf = wo.astype(ml_dtypes.bfloat16)
    # softmax shift-invariance: E = exp(sim - max(sim)) * exp(bias); the
    # row max over raw sim is enough (bias is O(5), no overflow risk).
    bias_bf = np.exp(bias).astype(ml_dtypes.bfloat16)

    in_maps = []
    for c in range(NCORES):
        sl = slice(RPC * c, RPC * (c + 1))
        in_maps.append(
            {
                "x": np.ascontiguousarray(x[:, sl, :]),
                "attn_bias": np.ascontiguousarray(bias_bf[:, sl, :]),
                "wq": wqp,
                "wk": wkp,
                "wv": wvp,
                "wo": wo_bf,
            }
        )
    return in_maps


def _run(inputs, trace=False, tmpdir=None):
    from concourse import bass_utils

    nc = _get_program()
    in_maps = _make_in_maps(inputs)
    res = bass_utils.run_bass_kernel_spmd(
        nc, in_maps, core_ids=list(range(NCORES)), trace=trace, tmpdir=tmpdir
    )
    outs = [np.asarray(res.results[c]["out"], np.float32) for c in range(NCORES)]
    return np.concatenate(outs, axis=1), res


def kernel(**inputs):
    out, _ = _run(inputs, trace=False)
    return out


def bench(inputs, iters=20, reps=1, ablate=()):
    """Steady-state timing of the compiled NEFF via PJRT with
    device-resident inputs. Returns (out_full, per_iter_seconds_list)."""
    import time

    import jax
    from jax.sharding import Mesh, PartitionSpec
    from jax.experimental.shard_map import shard_map
    from concourse import bass2jax, mybir

    nc = _get_program(reps, ablate)
    in_maps = _make_in_maps(inputs)

    partition_name = (
        nc.partition_id_tensor.name if nc.partition_id_tensor else None
    )
    in_names, out_names, out_avals, zero_outs = [], [], [], []
    for alloc in nc.m.functions[0].allocations:
        if not isinstance(alloc, mybir.MemoryLocationSet):
            continue
        name = alloc.memorylocations[0].name
        if alloc.kind == "ExternalInput":
            if name != partition_name:
                in_names.append(name)
        elif alloc.kind == "ExternalOutput":
            out_names.append(name)
            shape = tuple(alloc.tensor_shape)
            dtype = mybir.dt.np(alloc.dtype)
            out_avals.append(jax.core.ShapedArray(shape, dtype))
            zero_outs.append(np.zeros(shape, dtype))
    n_params = len(in_names)
    all_names = in_names + out_names
    if partition_name is not None:
        all_names.append(partition_name)

    chain = int(getattr(bench, "chain", 1))

    def _body(*args):
        ins = list(args[:n_params])
        zeros = list(args[n_params:])
        for _ in range(chain):
            operands = ins + zeros
            if partition_name is not None:
                operands.append(bass2jax.partition_id_tensor())
            outs = bass2jax._bass_exec_p.bind(
                *operands,
                out_avals=tuple(out_avals),
                in_names=tuple(all_names),
                out_names=tuple(out_names),
                lowering_input_output_aliases=(),
                sim_require_finite=True,
                sim_require_nnan=True,
                nc=nc,
            )
            zeros = list(outs)
        return tuple(outs)

    devices = jax.devices()[:NCORES]
    mesh = Mesh(np.asarray(devices), ("core",))
    n_outs = len(out_names)
    in_specs = (PartitionSpec("core"),) * (n_params + n_outs)
    out_specs = (PartitionSpec("core"),) * n_outs
    sharded = jax.jit(
        shard_map(
            _body, mesh=mesh, in_specs=in_specs, out_specs=out_specs,
            check_rep=False,
        ),
        keep_unused=True,
    )
    per_core = [
        [np.asarray(m[name]) for name in in_names] for m in in_maps
    ]
    concat_in = [
        np.concatenate([per_core[c][i] for c in range(NCORES)], axis=0)
        for i in range(n_params)
    ]
    concat_zeros = [
        np.zeros((NCORES * z.shape[0], *z.shape[1:]), z.dtype)
        for z in zero_outs
    ]
    from jax.sharding import NamedSharding

    shd = NamedSharding(mesh, PartitionSpec("core"))
    dev_in = [jax.device_put(a, shd) for a in concat_in]
    dev_zero = [jax.device_put(a, shd) for a in concat_zeros]
    jax.block_until_ready(dev_in)

    out = sharded(*dev_in, *dev_zero)
    jax.block_until_ready(out)
    times = []
    for _ in range(iters):
        t0 = time.perf_counter()
        out = sharded(*dev_in, *dev_zero)
        jax.block_until_ready(out)
        times.append(time.perf_counter() - t0)

    full = np.asarray(out[out_names.index("out")]).reshape(
        NCORES, B, RPC, D
    )
    full = np.concatenate([full[c] for c in range(NCORES)], axis=1)
    return full, times


if __name__ == "__main__":
    rng = np.random.default_rng(0)
    demo = {
        "x": rng.standard_normal((B, N, D), dtype=np.float32),
        "attn_bias": rng.standard_normal((H, N, N), dtype=np.float32),
        "gamma": np.ones((D,), np.float32),
        "wq": rng.standard_normal((D, H * DH), dtype=np.float32),
        "wk": rng.standard_normal((D, DH), dtype=np.float32),
        "wv": rng.standard_normal((D, DH), dtype=np.float32),
        "wo": rng.standard_normal((H * DH, D), dtype=np.float32),
    }
    out = kernel(**demo)
    print("out", out.shape, out.dtype, np.abs(out).mean())


# revision 47
# speedup vs baseline: 1.0021x; 1.0021x over previous
"""Distributed MQA attention kernel for 8 TRN2 NeuronCores.

Sharding: sequence-parallel over query rows. Core c owns query rows
[256c, 256(c+1)) of BOTH batches. All 16 heads stay local to each core, so
the output projection needs no cross-core reduction; the only collective is
a small AllGather of the shared (MQA) K^T / V projections.

Precision: logits have std ~2000 (un-normalized q·k), so softmax is
near-argmax: the q/k/scores path runs in true fp32 (4-pass matmuls).
attn@v and the output projection run in bf16.
"""

import sys

if "/opt/trn_rl_repo" not in sys.path:
    sys.path.insert(0, "/opt/trn_rl_repo")

import numpy as np

B = 2
N = 2048
D = 2048
H = 16
DH = 128
NCORES = 8
RPC = N // NCORES  # query rows per core, per batch (256)
EPS = 1e-5
SQRT_D = float(np.sqrt(np.float64(D)))

_PROGRAMS = {}


def _build_program(reps=1, ablate=()):
    ablate = set(ablate)
    from concourse import bacc, masks, mybir, tile

    f32 = mybir.dt.float32
    bf16 = mybir.dt.bfloat16
    Alu = mybir.AluOpType
    AFT = mybir.ActivationFunctionType

    nc = bacc.Bacc(
        "TRN2", target_bir_lowering=False, debug=False, num_devices=NCORES
    )

    x_d = nc.dram_tensor("x", (B, RPC, D), f32, kind="ExternalInput").ap()
    bias_d = nc.dram_tensor("attn_bias", (H, RPC, N), bf16, kind="ExternalInput").ap()
    wq_d = nc.dram_tensor("wq", (D, H * DH), f32, kind="ExternalInput").ap()
    wk_d = nc.dram_tensor("wk", (D, DH), f32, kind="ExternalInput").ap()
    wv_d = nc.dram_tensor("wv", (D, DH), f32, kind="ExternalInput").ap()
    wo_d = nc.dram_tensor("wo", (H * DH, D), bf16, kind="ExternalInput").ap()
    out_d = nc.dram_tensor("out", (B, RPC, D), f32, kind="ExternalOutput").ap()

    CT = D // 128  # 16 contraction tiles
    RB = (B * RPC) // 128  # 4 row blocks per core
    JT = N // 128  # 16 key tiles per batch

    with tile.TileContext(nc) as tc:
        with (
            tc.tile_pool(name="const", bufs=1) as const_pool,
            tc.tile_pool(name="persist", bufs=1) as persist,
            tc.tile_pool(name="xq", bufs=2) as xq_pool,
            tc.tile_pool(name="stat", bufs=12) as stat_pool,
            tc.tile_pool(name="bias", bufs=2) as bias_pool,
            tc.tile_pool(name="ebuf", bufs=3) as e_pool,
            tc.tile_pool(name="etbuf", bufs=3) as ets_pool,
            tc.tile_pool(name="vstg", bufs=1) as vstg_pool,
            tc.tile_pool(name="wo_s", bufs=2) as wo_pool,
            tc.tile_pool(name="otb", bufs=1) as ot_pool,
            tc.tile_pool(name="outb", bufs=2) as out_pool,
            tc.tile_pool(name="simps", bufs=2, space="PSUM") as sim_ps,
            tc.tile_pool(name="etps", bufs=1, space="PSUM") as et_ps,
            tc.tile_pool(name="shps", bufs=2, space="PSUM") as sh_ps,
            tc.tile_pool(name="dram", bufs=1, space="DRAM") as dram_pool,
        ):
            ident_f32 = const_pool.tile([128, 128], f32)
            ident_bf16 = const_pool.tile([128, 128], bf16)
            masks.make_identity(nc, ident_f32[:])
            masks.make_identity(nc, ident_bf16[:])

            qx3 = "no_qx3" not in ablate
            if not qx3:
                wk_sb = wo_pool.tile([128, CT, DH], f32, tag="wot", name="wk_sb")
                wv_sb = wo_pool.tile([128, CT, DH], f32, tag="wot", name="wv_sb")
                nc.sync.dma_start(
                    wk_sb[:], wk_d.rearrange("(ct p) d -> p ct d", p=128)
                )
                nc.sync.dma_start(
                    wv_sb[:], wv_d.rearrange("(ct p) d -> p ct d", p=128)
                )

            sc_bf = "scores_bf16" in ablate
            x3 = "x3" in ablate
            if qx3:
                uTs = persist.tile([128, CT, 2, B * RPC], bf16, name="uTs")
                uT = None
            else:
                uT = persist.tile([128, CT, B * RPC], f32)
                uTs = None
            if x3:
                qT = persist.tile([128, CT, 2, B * RPC], bf16)
                kT_full = persist.tile([128, B, N], f32)
                kT_use = persist.tile([128, B, 2, N], bf16)
            else:
                qT = persist.tile([128, CT, B * RPC], bf16 if sc_bf else f32)
                kT_full = persist.tile([128, B, N], f32)
                kT_use = persist.tile([128, B, N], bf16) if sc_bf else kT_full
            v_all = persist.tile([128, B * JT, DH], bf16)
            v_loc = persist.tile([128, RB, DH], f32)
            kT_loc = persist.tile([128, B * RPC], f32)

            bnc_k_in = dram_pool.tile([128, B * RPC], f32)
            bnc_k_out = dram_pool.tile([NCORES, 128, B * RPC], f32, addr_space="Shared")
            bnc_v_in = dram_pool.tile([B * RPC, DH], f32)
            bnc_v_out = dram_pool.tile([NCORES, B * RPC, DH], f32, addr_space="Shared")

            # ---- Stage 0: RMSNorm rows + transpose to u^T ------------------
            for b in range(B):
                for itile in range(RPC // 128):
                    rb = b * (RPC // 128) + itile
                    xt = xq_pool.tile([128, D], f32, tag="xq")
                    nc.sync.dma_start(xt[:], x_d[b, 128 * itile : 128 * (itile + 1), :])
                    scr = out_pool.tile([128, D], f32, tag="out_sb")
                    ssq = stat_pool.tile([128, 1], f32)
                    nc.scalar.activation(scr[:], xt[:], AFT.Square, accum_out=ssq[:])
                    tb = stat_pool.tile([128, 1], f32)
                    nc.vector.tensor_scalar_add(tb[:], ssq[:], EPS)
                    sq = stat_pool.tile([128, 1], f32)
                    nc.scalar.sqrt(sq[:], tb[:])
                    r0 = stat_pool.tile([128, 1], f32)
                    nc.vector.reciprocal(r0[:], sq[:])
                    # one Newton step for rsqrt accuracy: y = r0*(1.5-0.5*t*r0^2)
                    hh = stat_pool.tile([128, 1], f32)
                    nc.vector.tensor_mul(hh[:], r0[:], r0[:])
                    nc.vector.tensor_mul(hh[:], hh[:], tb[:])
                    h2 = stat_pool.tile([128, 1], f32)
                    nc.vector.tensor_scalar(h2[:], hh[:], -0.5, 1.5, Alu.mult, Alu.add)
                    sc = stat_pool.tile([128, 1], f32)
                    nc.vector.tensor_mul(sc[:], r0[:], h2[:])
                    nc.vector.tensor_scalar_mul(sc[:], sc[:], SQRT_D)
                    nc.vector.tensor_scalar_mul(xt[:], xt[:], sc[:])
                    for cg in range(4):
                        trp = sh_ps.tile([128, 512], f32, tag="shps")
                        for k in range(4):
                            ct = 4 * cg + k
                            nc.tensor.transpose(
                                trp[:, 128 * k : 128 * (k + 1)],
                                xt[:, 128 * ct : 128 * (ct + 1)],
                                ident_f32[:],
                            )
                        if qx3:
                            hi_sl = uTs[:, 4 * cg : 4 * cg + 4, 0,
                                        128 * rb : 128 * (rb + 1)]
                            nc.vector.tensor_copy(
                                hi_sl, trp.rearrange("p (k r) -> p k r", k=4)
                            )
                            nc.vector.tensor_sub(
                                uTs[:, 4 * cg : 4 * cg + 4, 1,
                                    128 * rb : 128 * (rb + 1)],
                                trp.rearrange("p (k r) -> p k r", k=4),
                                hi_sl,
                            )
                        else:
                            nc.vector.tensor_copy(
                                uT[:, 4 * cg : 4 * cg + 4, 128 * rb : 128 * (rb + 1)],
                                trp.rearrange("p (k r) -> p k r", k=4),
                            )

            # ---- Stage 1a: local k^T / v projections + AllGather -----------
            if qx3:
                wks = wo_pool.tile([128, CT, 2, DH], bf16, tag="wot", name="wks")
                wvs = wo_pool.tile([128, CT, DH], bf16, tag="wot", name="wvs")
                wk_tmp = xq_pool.tile([128, CT, DH], f32, tag="xq", name="wk_tmp")
                nc.sync.dma_start(
                    wk_tmp[:], wk_d.rearrange("(ct p) d -> p ct d", p=128)
                )
                nc.vector.tensor_copy(wks[:, :, 0, :], wk_tmp[:])
                nc.vector.tensor_sub(wks[:, :, 1, :], wk_tmp[:], wks[:, :, 0, :])
                wv_tmp = xq_pool.tile([128, CT, DH], f32, tag="xq", name="wv_tmp")
                nc.sync.dma_start(
                    wv_tmp[:], wv_d.rearrange("(ct p) d -> p ct d", p=128)
                )
                nc.vector.tensor_copy(wvs[:], wv_tmp[:])
            kps = sh_ps.tile([128, 512], f32, tag="shps")
            if qx3:
                for ct in range(CT):
                    nc.tensor.matmul(
                        kps[:], wks[:, ct, 0, :], uTs[:, ct, 0, :],
                        start=(ct == 0), stop=False,
                    )
                    nc.tensor.matmul(
                        kps[:], wks[:, ct, 0, :], uTs[:, ct, 1, :],
                        start=False, stop=False,
                    )
                    nc.tensor.matmul(
                        kps[:], wks[:, ct, 1, :], uTs[:, ct, 0, :],
                        start=False, stop=(ct == CT - 1),
                    )
            else:
                for ct in range(CT):
                    nc.tensor.matmul(
                        kps[:], wk_sb[:, ct, :], uT[:, ct, :],
                        start=(ct == 0), stop=(ct == CT - 1),
                    )
            nc.vector.tensor_copy(kT_loc[:], kps[:])
            nc.gpsimd.dma_start(bnc_k_in[:], kT_loc[:])

            for rb in range(RB):
                vps = sh_ps.tile([128, 128], f32, tag="shps")
                for ct in range(CT):
                    if qx3:
                        nc.tensor.matmul(
                            vps[:],
                            uTs[:, ct, 0, 128 * rb : 128 * (rb + 1)],
                            wvs[:, ct, :],
                            start=(ct == 0), stop=(ct == CT - 1),
                        )
                    else:
                        nc.tensor.matmul(
                            vps[:],
                            uT[:, ct, 128 * rb : 128 * (rb + 1)],
                            wv_sb[:, ct, :],
                            start=(ct == 0), stop=(ct == CT - 1),
                        )
                nc.vector.tensor_copy(v_loc[:, rb, :], vps[:])
            nc.gpsimd.dma_start(
                bnc_v_in.rearrange("(s p) d -> p s d", p=128), v_loc[:]
            )

            if "nocc" not in ablate:
                nc.gpsimd.collective_compute(
                    "AllGather", Alu.bypass,
                    replica_groups=[list(range(NCORES))],
                    ins=[bnc_k_in[:].opt()], outs=[bnc_k_out[:].opt()],
                )
                nc.gpsimd.collective_compute(
                    "AllGather", Alu.bypass,
                    replica_groups=[list(range(NCORES))],
                    ins=[bnc_v_in[:].opt()], outs=[bnc_v_out[:].opt()],
                )
            else:
                nc.gpsimd.dma_start(bnc_k_out[0], bnc_k_in[:])
                nc.gpsimd.dma_start(bnc_v_out[0], bnc_v_in[:])

            # ---- Stage 1b: q^T projection (fp32) ---------------------------
            for mt in range(H):
                wqt = xq_pool.tile([128, CT, 128], f32, tag="xq")
                nc.sync.dma_start(
                    wqt[:],
                    wq_d[:, 128 * mt : 128 * (mt + 1)].rearrange(
                        "(ct p) m -> p ct m", p=128
                    ),
                )
                qps = sh_ps.tile([128, 512], f32, tag="shps")
                if qx3:
                    wqs = xq_pool.tile([128, CT, 2, 128], bf16, tag="wqs", bufs=1)
                    for ct in range(CT):
                        nc.vector.tensor_copy(wqs[:, ct, 0, :], wqt[:, ct, :])
                        nc.vector.tensor_sub(
                            wqs[:, ct, 1, :], wqt[:, ct, :], wqs[:, ct, 0, :]
                        )
                    for ct in range(CT):
                        nc.tensor.matmul(
                            qps[:], wqs[:, ct, 0, :], uTs[:, ct, 0, :],
                            start=(ct == 0), stop=False,
                        )
                        nc.tensor.matmul(
                            qps[:], wqs[:, ct, 0, :], uTs[:, ct, 1, :],
                            start=False, stop=False,
                        )
                        nc.tensor.matmul(
                            qps[:], wqs[:, ct, 1, :], uTs[:, ct, 0, :],
                            start=False, stop=(ct == CT - 1),
                        )
                else:
                    for ct in range(CT):
                        nc.tensor.matmul(
                            qps[:], wqt[:, ct, :], uT[:, ct, :],
                            start=(ct == 0), stop=(ct == CT - 1),
                        )
                if x3:
                    nc.vector.tensor_copy(qT[:, mt, 0, :], qps[:])
                    nc.vector.tensor_sub(
                        qT[:, mt, 1, :], qps[:], qT[:, mt, 0, :],
                    )
                else:
                    nc.vector.tensor_copy(qT[:, mt, :], qps[:])

            # ---- gather-in: k^T full + v full ------------------------------
            # bounce layout: core g contributed cols [b0 rows | b1 rows]
            nc.sync.dma_start(
                kT_full.rearrange("p b (g r) -> p b g r", g=NCORES),
                bnc_k_out.rearrange("g p (b r) -> p b g r", b=B),
            )
            if sc_bf:
                nc.vector.tensor_copy(kT_use[:], kT_full[:])
            if x3:
                for b_ in range(B):
                    nc.vector.tensor_copy(kT_use[:, b_, 0, :], kT_full[:, b_, :])
                    nc.vector.tensor_sub(
                        kT_use[:, b_, 1, :], kT_full[:, b_, :],
                        kT_use[:, b_, 0, :],
                    )
            for g in range(NCORES):
                vst = out_pool.tile([128, RB, DH], f32, tag="out_sb", name="vst")
                nc.sync.dma_start(
                    vst[:], bnc_v_out[g].rearrange("(s p) d -> p s d", p=128)
                )
                for b in range(B):
                    nc.vector.tensor_copy(
                        v_all[:, b * JT + 2 * g : b * JT + 2 * g + 2, :],
                        vst[:, 2 * b : 2 * b + 2, :],
                    )

            # ---- Stage 2: attention + output projection --------------------
            def s2_front(itile, h, b, bt):
                """scores + softmax for one (itile, h, b); returns et2."""
                i0 = b * RPC + 128 * itile
                HW_ = N // 2  # half width (1024)
                et = e_pool.tile([128, N], bf16, tag="et")
                et2 = e_pool.tile([128, N], bf16, tag="et2")
                nms = stat_pool.tile([128, 2], f32, tag="nms")
                for hf in range(2):
                    simh = sim_ps.tile([128, HW_], f32, tag="sim")
                    for jb in range(2):
                        j0 = HW_ * hf + 512 * jb
                        if x3:
                            nc.tensor.matmul(
                                simh[:, 512 * jb : 512 * (jb + 1)],
                                qT[:, h, 0, i0 : i0 + 128],
                                kT_use[:, b, 0, j0 : j0 + 512],
                                start=True, stop=False,
                            )
                            nc.tensor.matmul(
                                simh[:, 512 * jb : 512 * (jb + 1)],
                                qT[:, h, 0, i0 : i0 + 128],
                                kT_use[:, b, 1, j0 : j0 + 512],
                                start=False, stop=False,
                            )
                            nc.tensor.matmul(
                                simh[:, 512 * jb : 512 * (jb + 1)],
                                qT[:, h, 1, i0 : i0 + 128],
                                kT_use[:, b, 0, j0 : j0 + 512],
                                start=False, stop=True,
                            )
                        else:
                            nc.tensor.matmul(
                                simh[:, 512 * jb : 512 * (jb + 1)],
                                qT[:, h, i0 : i0 + 128],
                                kT_use[:, b, j0 : j0 + 512],
                                start=True, stop=True,
                            )
                    # nms[:, hf] = -max(sim_half)
                    nc.vector.tensor_reduce(
                        out=nms[:, hf : hf + 1], in_=simh[:],
                        op=Alu.max, axis=mybir.AxisListType.X, negate=True,
                    )
                    nc.scalar.activation(
                        et[:, HW_ * hf : HW_ * (hf + 1)], simh[:],
                        AFT.Exp, bias=nms[:, hf : hf + 1], scale=1.0,
                    )
                # combined -max = min over halves; per-half rescale
                # corr_h = exp(nm - nm_h) folded into the stt scalar
                nm = stat_pool.tile([128, 1], f32, tag="nm")
                nc.vector.tensor_reduce(
                    out=nm[:], in_=nms[:], op=Alu.min, axis=mybir.AxisListType.X
                )
                dh = stat_pool.tile([128, 2], f32, tag="dh")
                nc.vector.tensor_scalar(
                    dh[:], nms[:], -1.0, nm[:], Alu.mult, Alu.add
                )
                corr = stat_pool.tile([128, 2], f32, tag="corr")
                nc.scalar.activation(corr[:], dh[:], AFT.Exp)
                rs2 = stat_pool.tile([128, 2], f32, tag="rs2")
                for hf in range(2):
                    nc.vector.scalar_tensor_tensor(
                        out=et2[:, HW_ * hf : HW_ * (hf + 1)],
                        in0=et[:, HW_ * hf : HW_ * (hf + 1)],
                        scalar=corr[:, hf : hf + 1],
                        in1=bt[:, HW_ * hf : HW_ * (hf + 1)],
                        op0=Alu.mult, op1=Alu.mult,
                        accum_out=rs2[:, hf : hf + 1],
                    )
                rs = stat_pool.tile([128, 1], f32, tag="rs")
                nc.vector.tensor_reduce(
                    out=rs[:], in_=rs2[:], op=Alu.add, axis=mybir.AxisListType.X
                )
                rcp = stat_pool.tile([128, 1], f32, tag="rcp")
                nc.vector.reciprocal(rcp[:], rs[:])
                nc.vector.tensor_scalar_mul(et2[:], et2[:], rcp[:])
                return et2

            def s2_back(b, h, et2, oT):
                """E-transpose + attn@v for one finished front iteration."""
                etp = et_ps.tile([128, N], bf16, tag="etp")
                for jt in range(JT):
                    nc.tensor.transpose(
                        etp[:, 128 * jt : 128 * (jt + 1)],
                        et2[:, 128 * jt : 128 * (jt + 1)],
                        ident_bf16[:],
                    )
                ets = ets_pool.tile([128, N], bf16, tag="ets")
                nc.scalar.copy(ets[:], etp[:])
                otp = sh_ps.tile([128, 128], f32, tag="shps")
                for jt in range(JT):
                    nc.tensor.matmul(
                        otp[:],
                        v_all[:, b * JT + jt, :],
                        ets[:, 128 * jt : 128 * (jt + 1)],
                        start=(jt == 0), stop=(jt == JT - 1),
                    )
                nc.vector.tensor_copy(oT[:, b, h, :], otp[:])

            def stage2():
                for itile in range(RPC // 128):
                    oT = ot_pool.tile([128, B, H, 128], bf16, tag="oT")
                    iters = [(h, b) for h in range(H) for b in range(B)]
                    LAG = 1
                    pending = []  # [(b, h, et2)] awaiting back-stage
                    bt = None
                    for idx in range(len(iters) + LAG):
                        if idx < len(iters):
                            h, b = iters[idx]
                            if b == 0:
                                # bt holds exp(bias) (host-precomputed)
                                bt = bias_pool.tile([128, N], bf16, tag="bt")
                                nc.sync.dma_start(
                                    bt[:],
                                    bias_d[h, 128 * itile : 128 * (itile + 1), :],
                                )
                            et2 = s2_front(itile, h, b, bt)
                            pending.append((b, h, et2))
                        if idx >= LAG:
                            pb, ph, pe2 = pending.pop(0)
                            s2_back(pb, ph, pe2, oT)
                    outs = []
                    for b in range(B):
                        ob = out_pool.tile([128, D], f32, tag="out_sb")
                        outs.append(ob)
                    for et_i in range(D // 256):
                        wot = wo_pool.tile([128, H, 256], bf16, tag="wot")
                        nc.sync.dma_start(
                            wot[:],
                            wo_d[:, 256 * et_i : 256 * (et_i + 1)].rearrange(
                                "(mt p) e -> p mt e", p=128
                            ),
                        )
                        for b in range(B):
                            fin = sh_ps.tile([128, 256], f32, tag="shps")
                            if "nowo" not in ablate:
                                for mt in range(H):
                                    nc.tensor.matmul(
                                        fin[:], oT[:, b, mt, :], wot[:, mt, :],
                                        start=(mt == 0), stop=(mt == H - 1),
                                    )
                            nc.vector.tensor_copy(
                                outs[b][:, 256 * et_i : 256 * (et_i + 1)], fin[:]
                            )
                    for b in range(B):
                        nc.sync.dma_start(
                            out_d[b, 128 * itile : 128 * (itile + 1), :], outs[b][:]
                        )

            if "skip2" in ablate:
                pass
            elif reps == 1:
                stage2()
            else:
                with tc.For_i(0, reps, 1):
                    stage2()

    nc.compile()
    return nc


def _get_program(reps=1, ablate=()):
    key = (reps, tuple(sorted(ablate)))
    if key not in _PROGRAMS:
        _PROGRAMS[key] = _build_program(reps, ablate)
    return _PROGRAMS[key]


def _make_in_maps(inputs):
    import ml_dtypes

    x = np.asarray(inputs["x"], np.float32)
    bias = np.asarray(inputs["attn_bias"], np.float32)
    gamma = np.asarray(inputs["gamma"], np.float32)
    wq = np.asarray(inputs["wq"], np.float32)
    wk = np.asarray(inputs["wk"], np.float32)
    wv = np.asarray(inputs["wv"], np.float32)
    wo = np.asarray(inputs["wo"], np.float32)

    g = gamma[:, None]
    wqp = np.ascontiguousarray(g * wq * np.float32(DH**-0.5), dtype=np.float32)
    wkp = np.ascontiguousarray(g * wk, dtype=np.float32)
    wvp = np.ascontiguousarray(g * wv, dtype=np.float32)
    wo_bf = wo.astype(ml_dtypes.bfloat16)
    # softmax shift-invariance: E = exp(sim - max(sim)) * exp(bias); the
    # row max over raw sim is enough (bias is O(5), no overflow risk).
    bias_bf = np.exp(bias).astype(ml_dtypes.bfloat16)

    in_maps = []
    for c in range(NCORES):
        sl = slice(RPC * c, RPC * (c + 1))
        in_maps.append(
            {
                "x": np.ascontiguousarray(x[:, sl, :]),
                "attn_bias": np.ascontiguousarray(bias_bf[:, sl, :]),
                "wq": wqp,
                "wk": wkp,
                "wv": wvp,
                "wo": wo_bf,
            }
        )
    return in_maps


def _run(inputs, trace=False, tmpdir=None):
    from concourse import bass_utils

    nc = _get_program()
    in_maps = _make_in_maps(inputs)
    res = bass_utils.run_bass_kernel_spmd(
        nc, in_maps, core_ids=list(range(NCORES)), trace=trace, tmpdir=tmpdir
    )
    outs = [np.asarray(res.results[c]["out"], np.float32) for c in range(NCORES)]
    return np.concatenate(outs, axis=1), res


def kernel(**inputs):
    out, _ = _run(inputs, trace=False)
    return out


def bench(inputs, iters=20, reps=1, ablate=()):
    """Steady-state timing of the compiled NEFF via PJRT with
    device-resident inputs. Returns (out_full, per_iter_seconds_list)."""
    import time

    import jax
    from jax.sharding import Mesh, PartitionSpec
    from jax.experimental.shard_map import shard_map
    from concourse import bass2jax, mybir

    nc = _get_program(reps, ablate)
    in_maps = _make_in_maps(inputs)

    partition_name = (
        nc.partition_id_tensor.name if nc.partition_id_tensor else None
    )
    in_names, out_names, out_avals, zero_outs = [], [], [], []
    for alloc in nc.m.functions[0].allocations:
        if not isinstance(alloc, mybir.MemoryLocationSet):
            continue
        name = alloc.memorylocations[0].name
        if alloc.kind == "ExternalInput":
            if name != partition_name:
                in_names.append(name)
        elif alloc.kind == "ExternalOutput":
            out_names.append(name)
            shape = tuple(alloc.tensor_shape)
            dtype = mybir.dt.np(alloc.dtype)
            out_avals.append(jax.core.ShapedArray(shape, dtype))
            zero_outs.append(np.zeros(shape, dtype))
    n_params = len(in_names)
    all_names = in_names + out_names
    if partition_name is not None:
        all_names.append(partition_name)

    chain = int(getattr(bench, "chain", 1))

    def _body(*args):
        ins = list(args[:n_params])
        zeros = list(args[n_params:])
        for _ in range(chain):
            operands = ins + zeros
            if partition_name is not None:
                operands.append(bass2jax.partition_id_tensor())
            outs = bass2jax._bass_exec_p.bind(
                *operands,
                out_avals=tuple(out_avals),
                in_names=tuple(all_names),
                out_names=tuple(out_names),
                lowering_input_output_aliases=(),
                sim_require_finite=True,
                sim_require_nnan=True,
                nc=nc,
            )
            zeros = list(outs)
        return tuple(outs)

    devices = jax.devices()[:NCORES]
    mesh = Mesh(np.asarray(devices), ("core",))
    n_outs = len(out_names)
    in_specs = (PartitionSpec("core"),) * (n_params + n_outs)
    out_specs = (PartitionSpec("core"),) * n_outs
    sharded = jax.jit(
        shard_map(
            _body, mesh=mesh, in_specs=in_specs, out_specs=out_specs,
            check_rep=False,
        ),
        keep_unused=True,
    )
    per_core = [
        [np.asarray(m[name]) for name in in_names] for m in in_maps
    ]
    concat_in = [
        np.concatenate([per_core[c][i] for c in range(NCORES)], axis=0)
        for i in range(n_params)
    ]
    concat_zeros = [
        np.zeros((NCORES * z.shape[0], *z.shape[1:]), z.dtype)
        for z in zero_outs
    ]
    from jax.sharding import NamedSharding

    shd = NamedSharding(mesh, PartitionSpec("core"))
    dev_in = [jax.device_put(a, shd) for a in concat_in]
    dev_zero = [jax.device_put(a, shd) for a in concat_zeros]
    jax.block_until_ready(dev_in)

    out = sharded(*dev_in, *dev_zero)
    jax.block_until_ready(out)
    times = []
    for _ in range(iters):
        t0 = time.perf_counter()
        out = sharded(*dev_in, *dev_zero)
        jax.block_until_ready(out)
        times.append(time.perf_counter() - t0)

    full = np.asarray(out[out_names.index("out")]).reshape(
        NCORES, B, RPC, D
    )
    full = np.concatenate([full[c] for c in range(NCORES)], axis=1)
    return full, times


if __name__ == "__main__":
    rng = np.random.default_rng(0)
    demo = {
        "x": rng.standard_normal((B, N, D), dtype=np.float32),
        "attn_bias": rng.standard_normal((H, N, N), dtype=np.float32),
        "gamma": np.ones((D,), np.float32),
        "wq": rng.standard_normal((D, H * DH), dtype=np.float32),
        "wk": rng.standard_normal((D, DH), dtype=np.float32),
        "wv": rng.standard_normal((D, DH), dtype=np.float32),
        "wo": rng.standard_normal((H * DH, D), dtype=np.float32),
    }
    out = kernel(**demo)
    print("out", out.shape, out.dtype, np.abs(out).mean())
